# revision 1
# baseline (speedup 1.0000x reference)
"""Trainium2 Bass kernel for nn_Att_mlp_softmax (GNN message passing).

Reference computation:
    e = relu(h @ W1 + b1) @ W2 + b2                       # [N, 1] per-node score
    att = softmax(where(G > 0, e.T broadcast, -9e15))     # row-wise over neighbors
    out = (G.sum(-1))[:, None] * (att @ h)                # degree-rescaled aggregation

Because the pre-softmax score of entry (i, j) depends only on column j, the
masked softmax collapses algebraically:
    att[i, j] = G[i, j] * w[j] / sum_j G[i, j] * w[j],  w = exp(e - c)
so with H' = [w * h | w | 1] (N x 130):
    Y = G @ H'
    out = Y[:, 129] * Y[:, :128] / Y[:, 128]
One big [N, N] x [N, 130] matmul replaces the N^2 softmax entirely.

Precision/perf: G is an exact 0/1 mask, so it is streamed in bf16 losslessly
(half the HBM traffic, 1 cycle/row matmul, FWL weight loads). H' is split
hi/lo into two bf16 columns per logical column (err ~2^-16), accumulated in
fp32 PSUM — near-fp32 accuracy at bf16 speed. H' chunks are built just-in-time
inside the main loop (3 fused DVE ops per chunk) so the build fully overlaps
the matmul stream.

Distribution: G is row-sharded across 8 NeuronCores (1024 rows each); h and the
MLP weights are replicated. Each core's G shard is passed pre-transposed
([N, 1024], contraction dim major) so SBUF tiles have the contraction dim on
partitions with fully contiguous DMA lines. h is passed twice: d-major bf16
hi/lo (hT2, for the MLP contraction) and chunk-major natural fp32 (hc, for the
H' build). No collectives needed.

Moving-operand layout per contraction chunk jc (259 bf16 columns):
    [0:128]  hi(w * h)    [128:256] lo(w * h)
    [256]    hi(w)        [257]     lo(w)        [258] ones
"""

import numpy as np

N = 8192
D = 128
HID = 64
N_CORES = 8
ROWS = N // N_CORES          # 1024 output rows per core
JC = N // 128                # 64 contraction chunks of 128
NCOL = 259                   # moving columns per chunk (hi|lo|w_hi|w_lo|1)
ESHIFT = -4.0                # exp(e - 4): cancels exactly in the ratio, keeps
                             # w in a comfortable fp32/bf16 range

_cache = {}


def _install_axon_hooks_shim():
    """Provide antenv.axon_hooks if the image lacks it (trn_boot step 6).

    concourse.bass_utils imports it unconditionally when BASS_TRACE is set;
    without the shim that import crashes instead of degrading.
    """
    import contextlib
    import ctypes
    import sys
    import types

    try:
        import antenv.axon_hooks  # noqa: F401
        return
    except ImportError:
        pass

    so_path = "/opt/axon/libaxon_pjrt.so"

    def _make_hook():
        try:
            lib = ctypes.CDLL(so_path)
        except OSError:
            return None
        if not hasattr(lib, "axon_start_nrt_profile"):
            return None
        lib.axon_start_nrt_profile.argtypes = [
            ctypes.POINTER(ctypes.c_int64),
            ctypes.c_size_t,
        ]
        lib.axon_start_nrt_profile.restype = ctypes.c_int64
        lib.axon_stop_nrt_profile.argtypes = [ctypes.c_char_p]
        lib.axon_stop_nrt_profile.restype = ctypes.c_int64

        @contextlib.contextmanager
        def _hook(output_dir, device_ids):
            import jax

            jax.devices()
            if device_ids:
                ids = (ctypes.c_int64 * len(device_ids))(*device_ids)
                rc = lib.axon_start_nrt_profile(ids, len(device_ids))
            else:
                rc = lib.axon_start_nrt_profile(None, 0)
            if rc != 0:
                raise RuntimeError(f"axon_start_nrt_profile rc={rc}")
            try:
                yield
            finally:
                lib.axon_stop_nrt_profile(str(output_dir).encode())

        return _hook

    mod = types.ModuleType("antenv.axon_hooks")
    _holder = {"hook": _make_hook()}
    mod.set_axon_ntff_profile_hook = lambda h: _holder.__setitem__("hook", h)
    mod.get_axon_ntff_profile_hook = lambda: _holder["hook"]
    sys.modules["antenv.axon_hooks"] = mod
    try:
        import antenv

        antenv.axon_hooks = mod
    except ImportError:
        pass


def build_nc(enable_asserts=False):
    """Build + compile the per-core Bass program (identical on all 8 cores)."""
    from concourse import bacc, mybir, tile

    f32 = mybir.dt.float32
    bf16 = mybir.dt.bfloat16
    AF = mybir.ActivationFunctionType
    ALU = mybir.AluOpType

    nc = bacc.Bacc(
        "TRN2",
        target_bir_lowering=False,
        debug=False,
        enable_asserts=enable_asserts,
        num_devices=N_CORES,
    )
    gT = nc.dram_tensor("gT", [N, ROWS], bf16, kind="ExternalInput").ap()
    # hT split hi/lo in bf16 (host-side) so the MLP matmuls run at bf16 speed;
    # packed as [D, 2, N] so one DMA covers both halves
    hT2 = nc.dram_tensor("hT2", [D, 2, N], bf16, kind="ExternalInput").ap()
    hc = nc.dram_tensor("hc", [128, JC, D], f32, kind="ExternalInput").ap()
    # W1 columns doubled so the MLP writes z onto BOTH psum partition halves;
    # the hi-cast lives on partitions 0-63 and the lo-residual on 64-127,
    # which lets ONE matmul per chunk contract hi and lo against a stacked W2.
    W1h = nc.dram_tensor("W1h", [D, 2 * HID], bf16, kind="ExternalInput").ap()
    W1l = nc.dram_tensor("W1l", [D, 2 * HID], bf16, kind="ExternalInput").ap()
    b1 = nc.dram_tensor("b1", [2 * HID, 1], f32, kind="ExternalInput").ap()
    # W2s rows 0-63: [W2_hi | W2_lo]; rows 64-127: [W2_hi | 0], so
    # aT2.T @ W2s = [aThi@W2hi + aTlo@W2hi | aThi@W2lo] column-pair-wise.
    W2s = nc.dram_tensor("W2s", [2 * HID, 2], bf16, kind="ExternalInput").ap()
    b2 = nc.dram_tensor("b2", [1, 1], f32, kind="ExternalInput").ap()
    out = nc.dram_tensor("out", [ROWS, D], f32, kind="ExternalOutput").ap()

    with tile.TileContext(nc) as tc:
        with (
            tc.tile_pool(name="const", bufs=1) as cpool,
            tc.tile_pool(name="big", bufs=1) as bigpool,
            tc.tile_pool(name="gbuf", bufs=3) as gpool,
            tc.tile_pool(name="hpbuf", bufs=16) as hpool,
            tc.tile_pool(name="outbuf", bufs=3) as opool,
            tc.tile_pool(name="small", bufs=2) as spool,
        ):
            W1h_sb = cpool.tile([D, 2 * HID], bf16)
            nc.sync.dma_start(W1h_sb[:], W1h[:])
            W1l_sb = cpool.tile([D, 2 * HID], bf16)
            nc.sync.dma_start(W1l_sb[:], W1l[:])
            b1_sb = cpool.tile([2 * HID, 1], f32)
            nc.sync.dma_start(b1_sb[:], b1[:])
            W2s_sb = cpool.tile([2 * HID, 2], bf16)
            nc.sync.dma_start(W2s_sb[:], W2s[:])
            b2_sb = cpool.tile([1, 1], f32)
            nc.sync.dma_start(b2_sb[:], b2[:])
            ones_row = cpool.tile([1, 128], f32)
            nc.vector.memset(ones_row[:], 1.0)

            # h DMAs go on the gpsimd (SWDGE) queue so their dispatch doesn't
            # serialize with the G stream on the sync (HWDGE) queue; the MLP
            # h input is chunked so compute starts after the first quarter.
            NHCH = 8
            hT2_sb = bigpool.tile([D, 2, N], bf16)
            for q in range(NHCH):
                sl = slice(q * (N // NHCH), (q + 1) * (N // NHCH))
                nc.gpsimd.dma_start(hT2_sb[:, :, sl], hT2[:, :, sl])
            hc_sb = bigpool.tile([128, JC, D], f32)
            nc.gpsimd.dma_start(hc_sb[:], hc[:])

            # aT2 rows 0-63: bf16 hi of relu(h@W1+b1); rows 64-127: bf16 lo
            aT2 = bigpool.tile([2 * HID, N], bf16)

            w_sb = cpool.tile([128, JC], f32)
            # wtail[:, :, jc] = [w_hi, w_lo, 1] for chunk jc
            wtail = cpool.tile([128, 3, JC], bf16)
            nc.vector.memset(wtail[:, 2, :], 1.0)
            w_rem = cpool.tile([128, JC], f32)

            with tc.tile_pool(name="ps_pre", bufs=2, space="PSUM") as ps_pre:
                # ~4us of dummy matmuls on a zero tile: trips the PE HAM
                # activity monitor out of its 1.2 GHz cold state before the
                # real MLP arrives (no input deps, runs during the h DMA)
                warm = cpool.tile([128, 512], bf16)
                nc.vector.memset(warm[:], 0.0)
                pwarm = ps_pre.tile([128, 512], f32, tag="pwarm")
                for _ in range(32):
                    nc.tensor.matmul(
                        pwarm[:], warm[:, 0:128], warm[:], start=True, stop=True
                    )

                # ebias = b2 + ESHIFT broadcast to 128 partitions via a K=1
                # matmul (avoids any slow single-partition ops)
                pb2 = ps_pre.tile([128, 1], f32, tag="pb2")
                nc.tensor.matmul(pb2[:], ones_row[:], b2_sb[:], start=True,
                                 stop=True)
                ebias_sb = cpool.tile([128, 1], f32)
                nc.vector.tensor_scalar_add(ebias_sb[:], pb2[:], ESHIFT)

                af_all = bigpool.tile([2 * HID, N], f32)
                for nb in range(N // 512):
                    # a = h @ W1 in 3 bf16 terms: hi*hi + hi*lo + lo*hi
                    # (the dropped lo*lo term is ~2^-32 relative), written to
                    # BOTH psum partition halves via the doubled W1 columns
                    pa = ps_pre.tile([2 * HID, 512], f32, tag="pa")
                    sl = slice(nb * 512, (nb + 1) * 512)
                    nc.tensor.matmul(
                        pa[:], W1h_sb[:], hT2_sb[:, 0, sl], start=True,
                        stop=False,
                    )
                    nc.tensor.matmul(
                        pa[:], W1l_sb[:], hT2_sb[:, 0, sl], start=False,
                        stop=False,
                    )
                    nc.tensor.matmul(
                        pa[:], W1h_sb[:], hT2_sb[:, 1, sl], start=False,
                        stop=True,
                    )
                    nc.scalar.activation(
                        af_all[:, sl], pa[:], AF.Relu, bias=b1_sb[:]
                    )
                    # bf16 hi on rows 0-63; lo residual computed entirely on
                    # rows 64-127 (in-place: cast first, then subtract from
                    # the fp32 copy), batched per 4 blocks
                    if nb % 4 == 3:
                        bl = slice((nb - 3) * 512, (nb + 1) * 512)
                        nc.vector.tensor_copy(aT2[:, bl], af_all[:, bl])
                        nc.vector.scalar_tensor_tensor(
                            aT2[HID:, bl], af_all[HID:, bl], 1.0,
                            aT2[HID:, bl],
                            op0=ALU.mult, op1=ALU.subtract,
                        )

                # e laid out [128, 64]: partition = j within chunk, column =
                # chunk. ONE bf16 matmul per chunk: the stacked aT2 contracts
                # hi (rows 0-63) and lo (rows 64-127) against the stacked W2s
                # into a column pair summed after:
                # e = (aThi@W2hi + aTlo@W2hi) + aThi@W2lo.
                pe2 = ps_pre.tile([128, JC, 2], f32, tag="pe2")
                pe2s = cpool.tile([128, JC, 2], f32)
                pe_sum = cpool.tile([128, JC], f32)
                # combine -> exp -> w hi/lo per quarter so the first H' chunks
                # unblock the main loop while later scores still compute
                QW = JC // 4
                for q in range(4):
                    for c in range(q * QW, (q + 1) * QW):
                        nc.tensor.matmul(
                            pe2[:, c, :],
                            aT2[:, c * 128 : (c + 1) * 128],
                            W2s_sb[:],
                            start=True,
                            stop=True,
                        )
                    ql = slice(q * QW, (q + 1) * QW)
                    nc.vector.tensor_copy(pe2s[:, ql, :], pe2[:, ql, :])
                    nc.vector.tensor_tensor(
                        pe_sum[:, ql], pe2s[:, ql, 0], pe2s[:, ql, 1],
                        op=ALU.add,
                    )
                    nc.scalar.activation(
                        w_sb[:, ql], pe_sum[:, ql], AF.Exp, bias=ebias_sb[:]
                    )
                    nc.vector.tensor_copy(wtail[:, 0, ql], w_sb[:, ql])
                    nc.vector.scalar_tensor_tensor(
                        w_rem[:, ql], w_sb[:, ql], 1.0, wtail[:, 0, ql],
                        op0=ALU.mult, op1=ALU.subtract,
                    )
                    nc.vector.tensor_copy(wtail[:, 1, ql], w_rem[:, ql])

            # Main accumulation: acc[it] [128, NCOL] += G_tile.T @ H'_chunk.
            gTr = gT.rearrange("(a p) i -> p a i", p=128)  # [128, JC, ROWS]
            with tc.tile_pool(name="ps_acc", bufs=8, space="PSUM") as ps_acc:
                accs = [
                    ps_acc.tile([128, NCOL], f32, tag="acc", name=f"acc{i}")
                    for i in range(8)
                ]
                GRP = 8  # contraction chunks per DMA (2 MB transfers)

                def build_hp(jc):
                    # just-in-time H' chunk build: 3 DVE ops
                    hp = hpool.tile([128, NCOL], bf16, tag="hp",
                                    name=f"hp{jc}")
                    nc.vector.tensor_scalar_mul(
                        hp[:, 0:128], hc_sb[:, jc, :], w_sb[:, jc : jc + 1]
                    )
                    nc.vector.scalar_tensor_tensor(
                        hp[:, 128:256], hc_sb[:, jc, :],
                        w_sb[:, jc : jc + 1], hp[:, 0:128],
                        op0=ALU.mult, op1=ALU.subtract,
                    )
                    nc.vector.tensor_copy(hp[:, 256:259], wtail[:, :, jc])
                    return hp

                for jg in range(JC // GRP - 1):
                    gt = gpool.tile([128, GRP, ROWS], bf16, tag="gt")
                    nc.sync.dma_start(
                        gt[:], gTr[:, jg * GRP : (jg + 1) * GRP, :]
                    )
                    for jci in range(GRP):
                        jc = jg * GRP + jci
                        hp = build_hp(jc)
                        for it in range(8):
                            nc.tensor.matmul(
                                accs[it][:],
                                gt[:, jci, it * 128 : (it + 1) * 128],
                                hp[:],
                                start=(jc == 0),
                                stop=False,
                            )

                # last group runs i-tile-major: bank it stops 8 matmuls after
                # bank it-1, so each bank's epilogue chain overlaps the
                # remaining banks' matmuls instead of serializing after all
                gt = gpool.tile([128, GRP, ROWS], bf16, tag="gt", name="gt_last")
                nc.sync.dma_start(gt[:], gTr[:, JC - GRP :, :])
                hps_last = [build_hp(JC - GRP + jci) for jci in range(GRP)]
                for it in range(8):
                    for jci in range(GRP):
                        nc.tensor.matmul(
                            accs[it][:],
                            gt[:, jci, it * 128 : (it + 1) * 128],
                            hps_last[jci][:],
                            start=False,
                            stop=(jci == GRP - 1),
                        )

                # epilogue, emitted per-bank-first so each bank's heavy DVE
                # ops run as soon as ITS accumulator stops (overlapping the
                # remaining banks' matmuls); the cross-bank r-chain and final
                # muls come last. (only one PSUM operand is legal per DVE op;
                # tails writes stay contiguous in the innermost dim —
                # non-contiguous DVE writes mis-lower on HW)
                tails = spool.tile([128, 8, 3], f32, tag="tails")
                shs = []
                for it in range(8):
                    nc.vector.tensor_copy(tails[:, it, :], accs[it][:, 256:259])
                    shlo = opool.tile([128, D], f32, tag="shlo",
                                      name=f"shlo{it}", bufs=2)
                    nc.vector.tensor_copy(shlo[:], accs[it][:, 128:256])
                    sh = opool.tile([128, D], f32, tag="sh", name=f"sh{it}",
                                    bufs=8)
                    nc.vector.tensor_tensor(
                        sh[:], accs[it][:, 0:128], shlo[:], op=ALU.add
                    )
                    shs.append(sh)
                swsum = spool.tile([128, 8], f32, tag="swsum")
                nc.vector.tensor_tensor(
                    swsum[:], tails[:, :, 0], tails[:, :, 1], op=ALU.add
                )
                swe = spool.tile([128, 8], f32, tag="swe")
                nc.vector.tensor_scalar_add(swe[:], swsum[:], 1e-30)
                rc8 = spool.tile([128, 8], f32, tag="rc8")
                nc.vector.reciprocal(rc8[:], swe[:])
                r8 = spool.tile([128, 8], f32, tag="r8")
                nc.vector.tensor_mul(r8[:], rc8[:], tails[:, :, 2])
                ot_all = opool.tile([128, 8, D], f32, tag="ot_all", bufs=1)
                for it in range(8):
                    nc.vector.tensor_scalar_mul(
                        ot_all[:, it, :], shs[it][:], r8[:, it : it + 1]
                    )
                nc.sync.dma_start(
                    out.rearrange("(a p) d -> p a d", p=128), ot_all[:]
                )

    nc.compile()
    return nc


def make_in_maps(graph_info, h, W1, b1, W2, b2):
    """Shard + lay out the full inputs for the 8 cores."""
    import ml_dtypes

    bf16 = ml_dtypes.bfloat16

    def hilo(x):
        hi = x.astype(bf16)
        lo = (x - hi.astype(np.float32)).astype(bf16)
        return hi, lo

    g = np.ascontiguousarray(graph_info, dtype=np.float32)
    GT = np.ascontiguousarray(g.T).astype(bf16)                # exact 0/1
    h = np.asarray(h, np.float32)
    hT = np.ascontiguousarray(h.T)                             # [D, N]
    hTh, hTl = hilo(hT)
    hT2 = np.ascontiguousarray(np.stack([hTh, hTl], axis=1))   # [D, 2, N]
    hcm = np.ascontiguousarray(
        h.reshape(JC, 128, D).transpose(1, 0, 2)               # [128, JC, D]
    )
    W1h, W1l = hilo(np.ascontiguousarray(np.asarray(W1, np.float32)))
    # doubled columns: the MLP psum carries z on both partition halves
    W1hd = np.ascontiguousarray(np.concatenate([W1h, W1h], axis=1))
    W1ld = np.ascontiguousarray(np.concatenate([W1l, W1l], axis=1))
    b1r = np.asarray(b1, np.float32).reshape(HID, 1)
    b1d = np.concatenate([b1r, b1r], axis=0)
    W2h, W2l = hilo(np.asarray(W2, np.float32).reshape(HID, 1))
    W2s = np.ascontiguousarray(
        np.concatenate(
            [np.concatenate([W2h, W2l], axis=1),
             np.concatenate([W2h, np.zeros_like(W2h)], axis=1)], axis=0
        )
    )
    b2r = np.asarray(b2, np.float32).reshape(1, 1)
    in_maps = []
    for c in range(N_CORES):
        in_maps.append(
            {
                "gT": GT[:, c * ROWS : (c + 1) * ROWS],
                "hT2": hT2,
                "hc": hcm,
                "W1h": W1hd,
                "W1l": W1ld,
                "b1": b1d,
                "W2s": W2s,
                "b2": b2r,
            }
        )
    return in_maps


def kernel(graph_info, h, W1, b1, W2, b2):
    _install_axon_hooks_shim()
    from concourse.bass_utils import run_bass_kernel_spmd

    if "nc" not in _cache:
        _cache["nc"] = build_nc()
    nc = _cache["nc"]

    in_maps = make_in_maps(graph_info, h, W1, b1, W2, b2)
    res = run_bass_kernel_spmd(nc, in_maps, list(range(N_CORES)))
    return np.concatenate([res.results[c]["out"] for c in range(N_CORES)], axis=0)



# revision 6
# speedup vs baseline: 1.4719x; 1.4719x over previous
"""Trainium2 Bass kernel for nn_Att_mlp_softmax (GNN message passing).

Reference computation:
    e = relu(h @ W1 + b1) @ W2 + b2                       # [N, 1] per-node score
    att = softmax(where(G > 0, e.T broadcast, -9e15))     # row-wise over neighbors
    out = (G.sum(-1))[:, None] * (att @ h)                # degree-rescaled aggregation

The pre-softmax score of entry (i, j) depends only on column j, so the masked
softmax collapses algebraically: with w = exp(e - c) and H' = [w * h | w | 1]
(N x 130):
    Y = G @ H'
    out = Y[:, 129] * Y[:, :128] / Y[:, 128]

Precision/perf (correctness gate is rel_err < 2e-2, so single 16-bit precision
is ample): G is an exact 0/1 mask streamed in fp8e4 (1 byte/elem HBM traffic,
FWL 4x weight loads); H' is fp16 (10-bit mantissa, upcast to e10m11 inside the
PE). PSUM accumulates fp32. Measured end-to-end rel err ~1e-4.

Distribution: G row-sharded across 8 NeuronCores (1024 rows each); h and MLP
weights replicated; no collectives. Each core's shard is passed pre-transposed
as gTr[p, a, i] = G[base+i, a*128+p] so stationary tiles are contiguous.

Pipeline (per 8-chunk group g, all phases overlap):
    PE:  z MMs for group g+2 (j-on-partitions MLP: stationary hT-tile, moving
         W1) interleave with the 64 main MMs of group g
    DVE: fused relu-dot chain (z+b1 -> relu*W2 -> reduce) for group g+1, then
         just-in-time H' builds for group g
    ACT: exp(e + b2 + ESHIFT) -> w per group
    DMA: G groups on sync queue, hT+consts on gpsimd queue, hc on scalar queue
The last group runs it-major so each PSUM bank's epilogue (deg/den rescale)
overlaps the remaining banks' matmuls.
"""

import numpy as np

N = 8192
D = 128
HID = 64
N_CORES = 8
ROWS = N // N_CORES          # 1024 output rows per core
JC = N // 128                # 64 contraction chunks of 128
GRP = 8                      # chunks per group (1 MB fp8 G DMA each)
NG = JC // GRP               # 8 groups
NCOL = 130                   # moving columns: [w*h (128) | w | 1]
ESHIFT = -4.0                # exp(e - 4): cancels exactly in the ratio, keeps
                             # w in fp16 range

_cache = {}


def _install_axon_hooks_shim():
    """Provide antenv.axon_hooks if the image lacks it (trn_boot step 6).

    concourse.bass_utils imports it unconditionally when BASS_TRACE is set;
    without the shim that import crashes instead of degrading.
    """
    import contextlib
    import ctypes
    import sys
    import types

    try:
        import antenv.axon_hooks  # noqa: F401
        return
    except ImportError:
        pass

    so_path = "/opt/axon/libaxon_pjrt.so"

    def _make_hook():
        try:
            lib = ctypes.CDLL(so_path)
        except OSError:
            return None
        if not hasattr(lib, "axon_start_nrt_profile"):
            return None
        lib.axon_start_nrt_profile.argtypes = [
            ctypes.POINTER(ctypes.c_int64),
            ctypes.c_size_t,
        ]
        lib.axon_start_nrt_profile.restype = ctypes.c_int64
        lib.axon_stop_nrt_profile.argtypes = [ctypes.c_char_p]
        lib.axon_stop_nrt_profile.restype = ctypes.c_int64

        @contextlib.contextmanager
        def _hook(output_dir, device_ids):
            import jax

            jax.devices()
            if device_ids:
                ids = (ctypes.c_int64 * len(device_ids))(*device_ids)
                rc = lib.axon_start_nrt_profile(ids, len(device_ids))
            else:
                rc = lib.axon_start_nrt_profile(None, 0)
            if rc != 0:
                raise RuntimeError(f"axon_start_nrt_profile rc={rc}")
            try:
                yield
            finally:
                lib.axon_stop_nrt_profile(str(output_dir).encode())

        return _hook

    mod = types.ModuleType("antenv.axon_hooks")
    _holder = {"hook": _make_hook()}
    mod.set_axon_ntff_profile_hook = lambda h: _holder.__setitem__("hook", h)
    mod.get_axon_ntff_profile_hook = lambda: _holder["hook"]
    sys.modules["antenv.axon_hooks"] = mod
    try:
        import antenv

        antenv.axon_hooks = mod
    except ImportError:
        pass


def build_nc(enable_asserts=False):
    """Build + compile the per-core Bass program (identical on all 8 cores)."""
    from concourse import bacc, mybir, tile

    f32 = mybir.dt.float32
    fp16 = mybir.dt.float16
    fp8 = mybir.dt.float8e4
    AF = mybir.ActivationFunctionType
    ALU = mybir.AluOpType
    AX = mybir.AxisListType

    nc = bacc.Bacc(
        "TRN2",
        target_bir_lowering=False,
        debug=False,
        enable_asserts=enable_asserts,
        num_devices=N_CORES,
    )
    gTr = nc.dram_tensor("gTr", [128, JC, ROWS], fp8, kind="ExternalInput").ap()
    hT = nc.dram_tensor("hT", [D, N], fp16, kind="ExternalInput").ap()
    hc = nc.dram_tensor("hc", [128, JC, D], fp16, kind="ExternalInput").ap()
    W1s = nc.dram_tensor("W1s", [D, HID], fp16, kind="ExternalInput").ap()
    b1bt = nc.dram_tensor("b1bt", [128, GRP, HID], fp16, kind="ExternalInput").ap()
    w2bt = nc.dram_tensor("w2bt", [128, GRP, HID], fp16, kind="ExternalInput").ap()
    ebias = nc.dram_tensor("ebias", [128, 1], f32, kind="ExternalInput").ap()
    out = nc.dram_tensor("out", [128, 8, D], fp16, kind="ExternalOutput").ap()

    with tile.TileContext(nc) as tc:
        with (
            tc.tile_pool(name="const", bufs=1) as cpool,
            tc.tile_pool(name="gbuf", bufs=4) as gpool,
            tc.tile_pool(name="hpbuf", bufs=16) as hpool,
            tc.tile_pool(name="sbuf", bufs=2) as spool,
            tc.tile_pool(name="outbuf", bufs=8) as opool,
            tc.tile_pool(name="ps_z", bufs=3, space="PSUM") as zpool,
            tc.tile_pool(name="ps_w", bufs=1, space="PSUM") as wpool,
            tc.tile_pool(name="ps_acc", bufs=3, space="PSUM") as ps_acc,
        ):
            # ---- constants / inputs ----
            W1s_sb = cpool.tile([D, HID], fp16)
            nc.gpsimd.dma_start(W1s_sb[:], W1s[:])
            b1bt_sb = cpool.tile([128, GRP, HID], fp16)
            nc.gpsimd.dma_start(b1bt_sb[:], b1bt[:])
            w2bt_sb = cpool.tile([128, GRP, HID], fp16)
            nc.gpsimd.dma_start(w2bt_sb[:], w2bt[:])
            ebias_sb = cpool.tile([128, 1], f32)
            nc.gpsimd.dma_start(ebias_sb[:], ebias[:])
            hT_sb = cpool.tile([D, N], fp16)
            for s in range(8):
                sl = slice(s * (N // 8), (s + 1) * (N // 8))
                nc.gpsimd.dma_start(hT_sb[:, sl], hT[:, sl])
            hc_sb = cpool.tile([128, JC, D], fp16)
            for q in range(4):
                sl = slice(q * (JC // 4), (q + 1) * (JC // 4))
                nc.scalar.dma_start(hc_sb[:, sl, :], hc[:, sl, :])

            # wones[:, 0, jc] = w_jc (written by ACT exp); [:, 1, jc] = 1.
            # Layout keeps every WRITE contiguous in the innermost dim
            # (non-contiguous engine writes mis-lower on HW); reads may stride.
            wones = cpool.tile([128, 2, JC], f32)
            nc.vector.memset(wones[:, 1, :], 1.0)

            # ---- PE warmup: trip the HAM out of its cold 1.2 GHz state while
            # the first hT slice streams in (no input deps) ----
            warm = cpool.tile([128, 128], fp16)
            nc.vector.memset(warm[:], 0.0)
            pwarm = wpool.tile([128, 128], f32, tag="pwarm")
            for _ in range(26):
                nc.tensor.matmul(pwarm[:], warm[:], warm[:], start=True, stop=True)

            # pack 3 accumulators per PSUM bank (3 * 130 f32 = 1560 B of 2 KB)
            acctiles = [
                ps_acc.tile([128, 3, NCOL], f32, tag="acc", name=f"accb{i}")
                for i in range(3)
            ]
            accs = [acctiles[i // 3][:, i % 3, :] for i in range(8)]

            gts = {}

            def emit_z(g):
                """MLP z for the 8 chunks of group g: z[j, k] on j-partitions."""
                gt = gpool.tile([128, GRP, ROWS], fp8, tag="gt", name=f"gt{g}")
                nc.sync.dma_start(gt[:], gTr[:, g * GRP : (g + 1) * GRP, :])
                gts[g] = gt
                zps = zpool.tile([128, GRP, HID], f32, tag="z", name=f"z{g}")
                for k in range(GRP):
                    c = g * GRP + k
                    nc.tensor.matmul(
                        zps[:, k, :],
                        hT_sb[:, c * 128 : (c + 1) * 128],
                        W1s_sb[:],
                        start=True,
                        stop=True,
                    )
                # fused relu-dot on DVE: e = sum_k relu(z + b1) * W2
                zb = spool.tile([128, GRP, HID], fp16, tag="zb")
                nc.vector.tensor_tensor(zb[:], zps[:], b1bt_sb[:], op=ALU.add)
                prod = spool.tile([128, GRP, HID], fp16, tag="prod")
                nc.vector.scalar_tensor_tensor(
                    prod[:], zb[:], 0.0, w2bt_sb[:], op0=ALU.max, op1=ALU.mult
                )
                e8 = spool.tile([128, GRP], f32, tag="e8")
                nc.vector.tensor_reduce(e8[:], prod[:], axis=AX.X, op=ALU.add)
                nc.scalar.activation(
                    wones[:, 0, g * GRP : (g + 1) * GRP], e8[:], AF.Exp,
                    bias=ebias_sb[:],
                )

            def build_hp(jc):
                """Just-in-time H' chunk: [w*h | w | 1] fp16."""
                hp = hpool.tile([128, NCOL], fp16, tag="hp", name=f"hp{jc}")
                nc.vector.tensor_scalar_mul(
                    hp[:, 0:128], hc_sb[:, jc, :], wones[:, 0, jc : jc + 1]
                )
                nc.vector.tensor_copy(hp[:, 128:130], wones[:, :, jc])
                return hp

            def epilogue(it):
                """out rows of bank it: deg/den rescale + store."""
                den = spool.tile([128, 1], f32, tag="den", name=f"den{it}", bufs=8)
                nc.vector.tensor_scalar_add(den[:], accs[it][:, 128:129], 1e-30)
                rc = spool.tile([128, 1], f32, tag="rc", name=f"rc{it}", bufs=8)
                nc.vector.reciprocal(rc[:], den[:])
                r = spool.tile([128, 1], f32, tag="r", name=f"r{it}", bufs=8)
                nc.vector.tensor_tensor(r[:], rc[:], accs[it][:, 129:130], op=ALU.mult)
                ot = opool.tile([128, D], fp16, tag="ot", name=f"ot{it}")
                nc.vector.tensor_scalar_mul(ot[:], accs[it][:, 0:128], r[:])
                nc.sync.dma_start(out[:, it, :], ot[:])

            emit_z(0)
            emit_z(1)
            for g in range(NG):
                if g + 2 < NG:
                    emit_z(g + 2)
                gt = gts.pop(g)
                if g < NG - 1:
                    for k in range(GRP):
                        jc = g * GRP + k
                        hp = build_hp(jc)
                        for it in range(8):
                            # start=True clears has_written for the WHOLE psum
                            # bank, so only the first slice sharing each bank
                            # may issue it; siblings then init via overwrite
                            # (has_written=0) on their first matmul.
                            nc.tensor.matmul(
                                accs[it][:],
                                gt[:, k, it * 128 : (it + 1) * 128],
                                hp[:],
                                start=(jc == 0 and it % 3 == 0),
                                stop=False,
                            )
                else:
                    # last group it-major: bank it stops 8 matmuls after bank
                    # it-1, so each bank's epilogue overlaps the rest
                    hps = [build_hp(g * GRP + k) for k in range(GRP)]
                    for it in range(8):
                        for k in range(GRP):
                            nc.tensor.matmul(
                                accs[it][:],
                                gt[:, k, it * 128 : (it + 1) * 128],
                                hps[k][:],
                                start=False,
                                stop=(k == GRP - 1),
                            )
                        epilogue(it)

    nc.compile()
    return nc


def make_in_maps(graph_info, h, W1, b1, W2, b2):
    """Shard + lay out the full inputs for the 8 cores."""
    import ml_dtypes

    fp16 = np.float16
    fp8 = ml_dtypes.float8_e4m3

    h = np.asarray(h, np.float32)
    hT = np.ascontiguousarray(h.T).astype(fp16)                # [D, N]
    hcm = np.ascontiguousarray(
        h.reshape(JC, 128, D).transpose(1, 0, 2)               # [128, JC, D]
    ).astype(fp16)
    W1s = np.asarray(W1, np.float32).astype(fp16)              # [D, HID]
    b1bt = np.ascontiguousarray(
        np.broadcast_to(np.asarray(b1, np.float32), (128, GRP, HID))
    ).astype(fp16)
    w2bt = np.ascontiguousarray(
        np.broadcast_to(np.asarray(W2, np.float32)[:, 0], (128, GRP, HID))
    ).astype(fp16)
    eb = np.full((128, 1), float(np.asarray(b2).reshape(-1)[0]) + ESHIFT,
                 np.float32)

    g8 = np.asarray(graph_info, np.float32).astype(fp8)        # exact 0/1
    in_maps = []
    for c in range(N_CORES):
        shard = g8[c * ROWS : (c + 1) * ROWS]                  # [1024, N]
        gTr = np.ascontiguousarray(
            shard.reshape(ROWS, JC, 128).transpose(2, 1, 0)    # [128, JC, 1024]
        )
        in_maps.append(
            {
                "gTr": gTr,
                "hT": hT,
                "hc": hcm,
                "W1s": W1s,
                "b1bt": b1bt,
                "w2bt": w2bt,
                "ebias": eb,
            }
        )
    return in_maps


def kernel(graph_info, h, W1, b1, W2, b2):
    _install_axon_hooks_shim()
    from concourse.bass_utils import run_bass_kernel_spmd

    if "nc" not in _cache:
        _cache["nc"] = build_nc()
    nc = _cache["nc"]

    in_maps = make_in_maps(graph_info, h, W1, b1, W2, b2)
    res = run_bass_kernel_spmd(nc, in_maps, list(range(N_CORES)))
    return np.concatenate(
        [
            res.results[c]["out"].transpose(1, 0, 2).reshape(ROWS, D)
            for c in range(N_CORES)
        ],
        axis=0,
    ).astype(np.float32)


# revision 8
# speedup vs baseline: 1.8716x; 1.2716x over previous
"""Trainium2 Bass kernel for nn_Att_mlp_softmax (GNN message passing).

Reference computation:
    e = relu(h @ W1 + b1) @ W2 + b2                       # [N, 1] per-node score
    att = softmax(where(G > 0, e.T broadcast, -9e15))     # row-wise over neighbors
    out = (G.sum(-1))[:, None] * (att @ h)                # degree-rescaled aggregation

The pre-softmax score of entry (i, j) depends only on column j, so the masked
softmax collapses algebraically: with w = exp(e - c) and H' = [w * h | w | 1]
(N x 130):
    Y = G @ H'
    out = Y[:, 129] * Y[:, :128] / Y[:, 128]

Precision/perf (correctness gate is rel_err < 2e-2, so single 16-bit precision
is ample): G is an exact 0/1 mask streamed in fp8e4 (1 byte/elem HBM traffic,
FWL 4x weight loads); H' is fp16 (10-bit mantissa, upcast to e10m11 inside the
PE). PSUM accumulates fp32. Measured end-to-end rel err ~1e-4.

Distribution: G row-sharded across 8 NeuronCores (1024 rows each); h and MLP
weights replicated; no collectives. Each core's shard is passed pre-transposed
as gTr[p, a, i] = G[base+i, a*128+p] so stationary tiles are contiguous.

Pipeline (per 8-chunk group g, all phases overlap):
    PE:  z MMs for group g+2 (j-on-partitions MLP: stationary hT-tile, moving
         W1) interleave with the 64 main MMs of group g
    DVE: fused relu-dot chain (z+b1 -> relu*W2 -> reduce) for group g+1, then
         just-in-time H' builds for group g
    ACT: exp(e + b2 + ESHIFT) -> w per group
    DMA: G groups on sync queue, hT+consts on gpsimd queue, hc on scalar queue
The last group runs it-major so each PSUM bank's epilogue (deg/den rescale)
overlaps the remaining banks' matmuls.
"""

import numpy as np

N = 8192
D = 128
HID = 64
N_CORES = 8
ROWS = N // N_CORES          # 1024 output rows per core
JC = N // 128                # 64 contraction chunks of 128
GRP = 8                      # chunks per group (1 MB fp8 G DMA each)
NG = JC // GRP               # 8 groups
NCOL = 130                   # moving columns: [w*h (128) | w | 1]
ESHIFT = -4.0                # exp(e - 4): cancels exactly in the ratio, keeps
                             # w in fp16 range

_cache = {}


def _install_axon_hooks_shim():
    """Provide antenv.axon_hooks if the image lacks it (trn_boot step 6).

    concourse.bass_utils imports it unconditionally when BASS_TRACE is set;
    without the shim that import crashes instead of degrading.
    """
    import contextlib
    import ctypes
    import sys
    import types

    try:
        import antenv.axon_hooks  # noqa: F401
        return
    except ImportError:
        pass

    so_path = "/opt/axon/libaxon_pjrt.so"

    def _make_hook():
        try:
            lib = ctypes.CDLL(so_path)
        except OSError:
            return None
        if not hasattr(lib, "axon_start_nrt_profile"):
            return None
        lib.axon_start_nrt_profile.argtypes = [
            ctypes.POINTER(ctypes.c_int64),
            ctypes.c_size_t,
        ]
        lib.axon_start_nrt_profile.restype = ctypes.c_int64
        lib.axon_stop_nrt_profile.argtypes = [ctypes.c_char_p]
        lib.axon_stop_nrt_profile.restype = ctypes.c_int64

        @contextlib.contextmanager
        def _hook(output_dir, device_ids):
            import jax

            jax.devices()
            if device_ids:
                ids = (ctypes.c_int64 * len(device_ids))(*device_ids)
                rc = lib.axon_start_nrt_profile(ids, len(device_ids))
            else:
                rc = lib.axon_start_nrt_profile(None, 0)
            if rc != 0:
                raise RuntimeError(f"axon_start_nrt_profile rc={rc}")
            try:
                yield
            finally:
                lib.axon_stop_nrt_profile(str(output_dir).encode())

        return _hook

    mod = types.ModuleType("antenv.axon_hooks")
    _holder = {"hook": _make_hook()}
    mod.set_axon_ntff_profile_hook = lambda h: _holder.__setitem__("hook", h)
    mod.get_axon_ntff_profile_hook = lambda: _holder["hook"]
    sys.modules["antenv.axon_hooks"] = mod
    try:
        import antenv

        antenv.axon_hooks = mod
    except ImportError:
        pass


def build_nc(enable_asserts=False):
    """Build + compile the per-core Bass program (identical on all 8 cores)."""
    from concourse import bacc, mybir, tile

    f32 = mybir.dt.float32
    fp16 = mybir.dt.float16
    fp8 = mybir.dt.float8e4
    AF = mybir.ActivationFunctionType
    ALU = mybir.AluOpType
    AX = mybir.AxisListType

    nc = bacc.Bacc(
        "TRN2",
        target_bir_lowering=False,
        debug=False,
        enable_asserts=enable_asserts,
        num_devices=N_CORES,
    )
    gTr = nc.dram_tensor("gTr", [128, JC, ROWS], fp8, kind="ExternalInput").ap()
    hT = nc.dram_tensor("hT", [D, N], fp16, kind="ExternalInput").ap()
    hc = nc.dram_tensor("hc", [128, JC, D], fp16, kind="ExternalInput").ap()
    W1s = nc.dram_tensor("W1s", [D, HID], fp16, kind="ExternalInput").ap()
    b1bt = nc.dram_tensor("b1bt", [128, GRP, HID], fp16, kind="ExternalInput").ap()
    w2bt = nc.dram_tensor("w2bt", [128, GRP, HID], fp16, kind="ExternalInput").ap()
    ebias = nc.dram_tensor("ebias", [128, 1], f32, kind="ExternalInput").ap()
    out = nc.dram_tensor("out", [128, 8, D], fp16, kind="ExternalOutput").ap()

    with tile.TileContext(nc) as tc:
        with (
            tc.tile_pool(name="const", bufs=1) as cpool,
            tc.tile_pool(name="gbuf", bufs=8) as gpool,
            tc.tile_pool(name="hpbuf", bufs=16) as hpool,
            tc.tile_pool(name="sbuf", bufs=2) as spool,
            tc.tile_pool(name="outbuf", bufs=8) as opool,
            tc.tile_pool(name="ps_z", bufs=3, space="PSUM") as zpool,
            tc.tile_pool(name="ps_w", bufs=1, space="PSUM") as wpool,
            tc.tile_pool(name="ps_acc", bufs=3, space="PSUM") as ps_acc,
        ):
            # ---- constants / inputs ----
            # Smalls go on the gpsimd queue; ALL heavy traffic goes on the
            # sync queue in exact need-order (hT for the z MMs two groups
            # ahead, then hc+G per group). The whole fp8 G shard is held in
            # SBUF (8 MB), so nothing ever waits on pool rotation and the
            # single queue's delivery order fully controls pacing.
            W1s_sb = cpool.tile([D, HID], fp16)
            nc.gpsimd.dma_start(W1s_sb[:], W1s[:])
            b1bt_sb = cpool.tile([128, GRP, HID], fp16)
            nc.gpsimd.dma_start(b1bt_sb[:], b1bt[:])
            w2bt_sb = cpool.tile([128, GRP, HID], fp16)
            nc.gpsimd.dma_start(w2bt_sb[:], w2bt[:])
            ebias_sb = cpool.tile([128, 1], f32)
            nc.gpsimd.dma_start(ebias_sb[:], ebias[:])

            hT_sb = cpool.tile([D, N], fp16)
            hc_sb = cpool.tile([128, JC, D], fp16)
            gts = {}

            def dma_hT(s):
                sl = slice(s * (N // 8), (s + 1) * (N // 8))
                nc.sync.dma_start(hT_sb[:, sl], hT[:, sl])

            def dma_group(g):
                sl = slice(g * GRP, (g + 1) * GRP)
                nc.sync.dma_start(hc_sb[:, sl, :], hc[:, sl, :])
                gt = gpool.tile([128, GRP, ROWS], fp8, tag="gt", name=f"gt{g}")
                nc.sync.dma_start(gt[:], gTr[:, sl, :])
                gts[g] = gt

            dma_hT(0)
            dma_hT(1)
            dma_hT(2)
            for g in range(NG):
                dma_group(g)
                if g + 3 < NG:
                    dma_hT(g + 3)

            # wones[:, 0, jc] = w_jc (written by ACT exp); [:, 1, jc] = 1.
            # Layout keeps every WRITE contiguous in the innermost dim
            # (non-contiguous engine writes mis-lower on HW); reads may stride.
            wones = cpool.tile([128, 2, JC], f32)
            nc.vector.memset(wones[:, 1, :], 1.0)

            # ---- PE warmup: trip the HAM out of its cold 1.2 GHz state while
            # the first hT slice streams in (no input deps) ----
            warm = cpool.tile([128, 128], fp16)
            nc.vector.memset(warm[:], 0.0)
            pwarm = wpool.tile([128, 128], f32, tag="pwarm")
            for _ in range(26):
                nc.tensor.matmul(pwarm[:], warm[:], warm[:], start=True, stop=True)

            # pack 3 accumulators per PSUM bank (3 * 130 f32 = 1560 B of 2 KB)
            acctiles = [
                ps_acc.tile([128, 3, NCOL], f32, tag="acc", name=f"accb{i}")
                for i in range(3)
            ]
            accs = [acctiles[i // 3][:, i % 3, :] for i in range(8)]

            def emit_z(g):
                """MLP z for the 8 chunks of group g: z[j, k] on j-partitions."""
                zps = zpool.tile([128, GRP, HID], f32, tag="z", name=f"z{g}")
                for k in range(GRP):
                    c = g * GRP + k
                    nc.tensor.matmul(
                        zps[:, k, :],
                        hT_sb[:, c * 128 : (c + 1) * 128],
                        W1s_sb[:],
                        start=True,
                        stop=True,
                    )
                # fused relu-dot on DVE: e = sum_k relu(z + b1) * W2
                zb = spool.tile([128, GRP, HID], fp16, tag="zb")
                nc.vector.tensor_tensor(zb[:], zps[:], b1bt_sb[:], op=ALU.add)
                prod = spool.tile([128, GRP, HID], fp16, tag="prod")
                nc.vector.scalar_tensor_tensor(
                    prod[:], zb[:], 0.0, w2bt_sb[:], op0=ALU.max, op1=ALU.mult
                )
                e8 = spool.tile([128, GRP], f32, tag="e8")
                nc.vector.tensor_reduce(e8[:], prod[:], axis=AX.X, op=ALU.add)
                nc.scalar.activation(
                    wones[:, 0, g * GRP : (g + 1) * GRP], e8[:], AF.Exp,
                    bias=ebias_sb[:],
                )

            def build_hp(jc):
                """Just-in-time H' chunk: [w*h | w | 1] fp16."""
                hp = hpool.tile([128, NCOL], fp16, tag="hp", name=f"hp{jc}")
                nc.vector.tensor_scalar_mul(
                    hp[:, 0:128], hc_sb[:, jc, :], wones[:, 0, jc : jc + 1]
                )
                nc.vector.tensor_copy(hp[:, 128:130], wones[:, :, jc])
                return hp

            def epilogue(it):
                """out rows of bank it: deg/den rescale + store."""
                den = spool.tile([128, 1], f32, tag="den", name=f"den{it}", bufs=8)
                nc.vector.tensor_scalar_add(den[:], accs[it][:, 128:129], 1e-30)
                rc = spool.tile([128, 1], f32, tag="rc", name=f"rc{it}", bufs=8)
                nc.vector.reciprocal(rc[:], den[:])
                r = spool.tile([128, 1], f32, tag="r", name=f"r{it}", bufs=8)
                nc.vector.tensor_tensor(r[:], rc[:], accs[it][:, 129:130], op=ALU.mult)
                ot = opool.tile([128, D], fp16, tag="ot", name=f"ot{it}")
                nc.vector.tensor_scalar_mul(ot[:], accs[it][:, 0:128], r[:])
                nc.sync.dma_start(out[:, it, :], ot[:])

            emit_z(0)
            emit_z(1)
            for g in range(NG):
                if g + 2 < NG:
                    emit_z(g + 2)
                gt = gts.pop(g)

                if g < NG - 1:
                    for k in range(GRP):
                        jc = g * GRP + k
                        hp = build_hp(jc)
                        for it in range(8):
                            # start=True clears has_written for the WHOLE psum
                            # bank, so only the first slice sharing each bank
                            # may issue it; siblings then init via overwrite
                            # (has_written=0) on their first matmul.
                            nc.tensor.matmul(
                                accs[it][:],
                                gt[:, k, it * 128 : (it + 1) * 128],
                                hp[:],
                                start=(jc == 0 and it % 3 == 0),
                                stop=False,
                            )
                else:
                    # last group it-major: each bank's epilogue overlaps the
                    # remaining banks' matmuls. Bank order hops across PSUM
                    # banks (accs share banks in triples) so an epilogue's
                    # DVE reads never WAR-block the next bank's matmuls.
                    hps = [build_hp(g * GRP + k) for k in range(GRP)]
                    for it in (0, 3, 6, 1, 4, 7, 2, 5):
                        for k in range(GRP):
                            nc.tensor.matmul(
                                accs[it][:],
                                gt[:, k, it * 128 : (it + 1) * 128],
                                hps[k][:],
                                start=False,
                                stop=(k == GRP - 1),
                            )
                        epilogue(it)

    nc.compile()
    return nc


def make_in_maps(graph_info, h, W1, b1, W2, b2):
    """Shard + lay out the full inputs for the 8 cores."""
    import ml_dtypes

    fp16 = np.float16
    fp8 = ml_dtypes.float8_e4m3

    h = np.asarray(h, np.float32)
    hT = np.ascontiguousarray(h.T).astype(fp16)                # [D, N]
    hcm = np.ascontiguousarray(
        h.reshape(JC, 128, D).transpose(1, 0, 2)               # [128, JC, D]
    ).astype(fp16)
    W1s = np.asarray(W1, np.float32).astype(fp16)              # [D, HID]
    b1bt = np.ascontiguousarray(
        np.broadcast_to(np.asarray(b1, np.float32), (128, GRP, HID))
    ).astype(fp16)
    w2bt = np.ascontiguousarray(
        np.broadcast_to(np.asarray(W2, np.float32)[:, 0], (128, GRP, HID))
    ).astype(fp16)
    eb = np.full((128, 1), float(np.asarray(b2).reshape(-1)[0]) + ESHIFT,
                 np.float32)

    g8 = np.asarray(graph_info, np.float32).astype(fp8)        # exact 0/1
    in_maps = []
    for c in range(N_CORES):
        shard = g8[c * ROWS : (c + 1) * ROWS]                  # [1024, N]
        gTr = np.ascontiguousarray(
            shard.reshape(ROWS, JC, 128).transpose(2, 1, 0)    # [128, JC, 1024]
        )
        in_maps.append(
            {
                "gTr": gTr,
                "hT": hT,
                "hc": hcm,
                "W1s": W1s,
                "b1bt": b1bt,
                "w2bt": w2bt,
                "ebias": eb,
            }
        )
    return in_maps


def kernel(graph_info, h, W1, b1, W2, b2):
    _install_axon_hooks_shim()
    from concourse.bass_utils import run_bass_kernel_spmd

    if "nc" not in _cache:
        _cache["nc"] = build_nc()
    nc = _cache["nc"]

    in_maps = make_in_maps(graph_info, h, W1, b1, W2, b2)
    res = run_bass_kernel_spmd(nc, in_maps, list(range(N_CORES)))
    return np.concatenate(
        [
            res.results[c]["out"].transpose(1, 0, 2).reshape(ROWS, D)
            for c in range(N_CORES)
        ],
        axis=0,
    ).astype(np.float32)


# revision 10
# speedup vs baseline: 1.9192x; 1.0254x over previous
"""Trainium2 Bass kernel for nn_Att_mlp_softmax (GNN message passing).

Reference computation:
    e = relu(h @ W1 + b1) @ W2 + b2                       # [N, 1] per-node score
    att = softmax(where(G > 0, e.T broadcast, -9e15))     # row-wise over neighbors
    out = (G.sum(-1))[:, None] * (att @ h)                # degree-rescaled aggregation

The pre-softmax score of entry (i, j) depends only on column j, so the masked
softmax collapses algebraically: with w = exp(e - c) and H' = [w * h | w | 1]
(N x 130):
    Y = G @ H'
    out = Y[:, 129] * Y[:, :128] / Y[:, 128]

Precision/perf (correctness gate is rel_err < 2e-2, so single 16-bit precision
is ample): G is an exact 0/1 mask streamed in fp8e4 (1 byte/elem HBM traffic,
FWL 4x weight loads); H' is fp16 (10-bit mantissa, upcast to e10m11 inside the
PE). PSUM accumulates fp32. Measured end-to-end rel err ~1e-4.

Distribution: G row-sharded across 8 NeuronCores (1024 rows each); h and MLP
weights replicated; no collectives. Each core's shard is passed pre-transposed
as gTr[p, a, i] = G[base+i, a*128+p] so stationary tiles are contiguous.

Pipeline (per 8-chunk group g, all phases overlap):
    PE:  z MMs for group g+2 (j-on-partitions MLP: stationary hT-tile, moving
         W1) interleave with the 64 main MMs of group g
    DVE: fused relu-dot chain (z+b1 -> relu*W2 -> reduce) for group g+1, then
         just-in-time H' builds for group g
    ACT: exp(e + b2 + ESHIFT) -> w per group
    DMA: G groups on sync queue, hT+consts on gpsimd queue, hc on scalar queue
The last group runs it-major so each PSUM bank's epilogue (deg/den rescale)
overlaps the remaining banks' matmuls.
"""

import numpy as np

N = 8192
D = 128
HID = 64
N_CORES = 8
ROWS = N // N_CORES          # 1024 output rows per core
JC = N // 128                # 64 contraction chunks of 128
GRP = 8                      # chunks per group (1 MB fp8 G DMA each)
NG = JC // GRP               # 8 groups
NCOL = 130                   # moving columns: [w*h (128) | w | 1]
ESHIFT = -4.0                # exp(e - 4): cancels exactly in the ratio, keeps
                             # w in fp16 range

_cache = {}


def _install_axon_hooks_shim():
    """Provide antenv.axon_hooks if the image lacks it (trn_boot step 6).

    concourse.bass_utils imports it unconditionally when BASS_TRACE is set;
    without the shim that import crashes instead of degrading.
    """
    import contextlib
    import ctypes
    import sys
    import types

    try:
        import antenv.axon_hooks  # noqa: F401
        return
    except ImportError:
        pass

    so_path = "/opt/axon/libaxon_pjrt.so"

    def _make_hook():
        try:
            lib = ctypes.CDLL(so_path)
        except OSError:
            return None
        if not hasattr(lib, "axon_start_nrt_profile"):
            return None
        lib.axon_start_nrt_profile.argtypes = [
            ctypes.POINTER(ctypes.c_int64),
            ctypes.c_size_t,
        ]
        lib.axon_start_nrt_profile.restype = ctypes.c_int64
        lib.axon_stop_nrt_profile.argtypes = [ctypes.c_char_p]
        lib.axon_stop_nrt_profile.restype = ctypes.c_int64

        @contextlib.contextmanager
        def _hook(output_dir, device_ids):
            import jax

            jax.devices()
            if device_ids:
                ids = (ctypes.c_int64 * len(device_ids))(*device_ids)
                rc = lib.axon_start_nrt_profile(ids, len(device_ids))
            else:
                rc = lib.axon_start_nrt_profile(None, 0)
            if rc != 0:
                raise RuntimeError(f"axon_start_nrt_profile rc={rc}")
            try:
                yield
            finally:
                lib.axon_stop_nrt_profile(str(output_dir).encode())

        return _hook

    mod = types.ModuleType("antenv.axon_hooks")
    _holder = {"hook": _make_hook()}
    mod.set_axon_ntff_profile_hook = lambda h: _holder.__setitem__("hook", h)
    mod.get_axon_ntff_profile_hook = lambda: _holder["hook"]
    sys.modules["antenv.axon_hooks"] = mod
    try:
        import antenv

        antenv.axon_hooks = mod
    except ImportError:
        pass


def build_nc(enable_asserts=False):
    """Build + compile the per-core Bass program (identical on all 8 cores)."""
    from concourse import bacc, mybir, tile

    f32 = mybir.dt.float32
    fp16 = mybir.dt.float16
    fp8 = mybir.dt.float8e4
    AF = mybir.ActivationFunctionType
    ALU = mybir.AluOpType
    AX = mybir.AxisListType

    nc = bacc.Bacc(
        "TRN2",
        target_bir_lowering=False,
        debug=False,
        enable_asserts=enable_asserts,
        num_devices=N_CORES,
    )
    gTr = nc.dram_tensor("gTr", [128, JC, ROWS], fp8, kind="ExternalInput").ap()
    hT = nc.dram_tensor("hT", [D, N], fp16, kind="ExternalInput").ap()
    hc = nc.dram_tensor("hc", [128, JC, D + 1], fp16, kind="ExternalInput").ap()
    W1s = nc.dram_tensor("W1s", [D, HID], fp16, kind="ExternalInput").ap()
    b1bt = nc.dram_tensor("b1bt", [128, GRP, HID], fp16, kind="ExternalInput").ap()
    w2bt = nc.dram_tensor("w2bt", [128, GRP, HID], fp16, kind="ExternalInput").ap()
    ebias = nc.dram_tensor("ebias", [128, 1], f32, kind="ExternalInput").ap()
    out = nc.dram_tensor("out", [128, 8, D], fp16, kind="ExternalOutput").ap()

    with tile.TileContext(nc) as tc:
        with (
            tc.tile_pool(name="const", bufs=1) as cpool,
            tc.tile_pool(name="gbuf", bufs=8) as gpool,
            tc.tile_pool(name="hpbuf", bufs=16) as hpool,
            tc.tile_pool(name="sbuf", bufs=2) as spool,
            tc.tile_pool(name="outbuf", bufs=1) as opool,
            tc.tile_pool(name="ps_z", bufs=3, space="PSUM") as zpool,
            tc.tile_pool(name="ps_w", bufs=1, space="PSUM") as wpool,
            tc.tile_pool(name="ps_acc", bufs=3, space="PSUM") as ps_acc,
        ):
            # ---- constants / inputs ----
            # Smalls go on the gpsimd queue; ALL heavy traffic goes on the
            # sync queue in exact need-order (hT for the z MMs two groups
            # ahead, then hc+G per group). The whole fp8 G shard is held in
            # SBUF (8 MB), so nothing ever waits on pool rotation and the
            # single queue's delivery order fully controls pacing.
            W1s_sb = cpool.tile([D, HID], fp16)
            nc.scalar.dma_start(W1s_sb[:], W1s[:])
            b1bt_sb = cpool.tile([128, GRP, HID], fp16)
            nc.scalar.dma_start(b1bt_sb[:], b1bt[:])
            w2bt_sb = cpool.tile([128, GRP, HID], fp16)
            nc.scalar.dma_start(w2bt_sb[:], w2bt[:])
            ebias_sb = cpool.tile([128, 1], f32)
            nc.scalar.dma_start(ebias_sb[:], ebias[:])

            hT_sb = cpool.tile([D, N], fp16)
            hc_sb = cpool.tile([128, JC, D + 1], fp16)
            gts = {}

            def dma_hT(s):
                sl = slice(s * (N // 8), (s + 1) * (N // 8))
                nc.sync.dma_start(hT_sb[:, sl], hT[:, sl])

            def dma_group(g):
                sl = slice(g * GRP, (g + 1) * GRP)
                nc.sync.dma_start(hc_sb[:, sl, :], hc[:, sl, :])
                gt = gpool.tile([128, GRP, ROWS], fp8, tag="gt", name=f"gt{g}")
                nc.sync.dma_start(gt[:], gTr[:, sl, :])
                gts[g] = gt

            dma_hT(0)
            dma_hT(1)
            for g in range(NG):
                dma_group(g)
                if g + 2 < NG:
                    dma_hT(g + 2)

            # preset the constant ones column (col 129) in every hp ring
            # buffer once; later ts_mul builds write only cols 0:129
            for b in range(16):
                t = hpool.tile([128, NCOL], fp16, tag="hp", name=f"hpinit{b}")
                nc.vector.memset(t[:, 129:130], 1.0)

            # wones[:, 0, jc] = w_jc (written by ACT exp); [:, 1, jc] = 1.
            # Layout keeps every WRITE contiguous in the innermost dim
            # (non-contiguous engine writes mis-lower on HW); reads may stride.
            wones = cpool.tile([128, 2, JC], f32)

            # ---- PE warmup: trip the HAM out of its cold 1.2 GHz state while
            # the first hT slice streams in (no input deps) ----
            warm = cpool.tile([128, 128], fp16)
            nc.vector.memset(warm[:], 0.0)
            pwarm = wpool.tile([128, 128], f32, tag="pwarm")
            for _ in range(26):
                nc.tensor.matmul(pwarm[:], warm[:], warm[:], start=True, stop=True)

            # pack 3 accumulators per PSUM bank (3 * 130 f32 = 1560 B of 2 KB)
            acctiles = [
                ps_acc.tile([128, 3, NCOL], f32, tag="acc", name=f"accb{i}")
                for i in range(3)
            ]
            accs = [acctiles[i // 3][:, i % 3, :] for i in range(8)]

            def emit_z(g):
                """MLP z for the 8 chunks of group g: z[j, k] on j-partitions."""
                zps = zpool.tile([128, GRP, HID], f32, tag="z", name=f"z{g}")
                for k in range(GRP):
                    c = g * GRP + k
                    nc.tensor.matmul(
                        zps[:, k, :],
                        hT_sb[:, c * 128 : (c + 1) * 128],
                        W1s_sb[:],
                        start=True,
                        stop=True,
                    )
                # fused relu-dot on DVE: e = sum_k relu(z + b1) * W2
                zb = spool.tile([128, GRP, HID], fp16, tag="zb")
                nc.vector.tensor_tensor(zb[:], zps[:], b1bt_sb[:], op=ALU.add)
                prod = spool.tile([128, GRP, HID], fp16, tag="prod")
                nc.vector.scalar_tensor_tensor(
                    prod[:], zb[:], 0.0, w2bt_sb[:], op0=ALU.max, op1=ALU.mult
                )
                e8 = spool.tile([128, GRP], f32, tag="e8")
                nc.vector.tensor_reduce(e8[:], prod[:], axis=AX.X, op=ALU.add)
                nc.scalar.activation(
                    wones[:, 0, g * GRP : (g + 1) * GRP], e8[:], AF.Exp,
                    bias=ebias_sb[:],
                )

            def build_hp(jc):
                """Just-in-time H' chunk: [w*h | w | 1] fp16.

                hc carries a host-side ones column, so ONE ts_mul produces
                [w*h | w]; the constant 1 in col 129 was preset per buffer.
                """
                hp = hpool.tile([128, NCOL], fp16, tag="hp", name=f"hp{jc}")
                nc.vector.tensor_scalar_mul(
                    hp[:, 0:129], hc_sb[:, jc, :], wones[:, 0, jc : jc + 1]
                )
                return hp

            def epilogue(it):
                """out rows of bank it: deg/den rescale + store."""
                den = spool.tile([128, 1], f32, tag="den", name=f"den{it}", bufs=8)
                nc.vector.tensor_scalar_add(den[:], accs[it][:, 128:129], 1e-30)
                rc = spool.tile([128, 1], f32, tag="rc", name=f"rc{it}", bufs=8)
                nc.vector.reciprocal(rc[:], den[:])
                r = spool.tile([128, 1], f32, tag="r", name=f"r{it}", bufs=8)
                nc.vector.tensor_tensor(r[:], rc[:], accs[it][:, 129:130], op=ALU.mult)
                nc.vector.tensor_scalar_mul(
                    ot_all[:, it, :], accs[it][:, 0:128], r[:]
                )

            ot_all = opool.tile([128, 8, D], fp16, tag="ot_all", bufs=1)

            emit_z(0)
            emit_z(1)
            for g in range(NG):
                if g + 2 < NG:
                    emit_z(g + 2)
                gt = gts.pop(g)

                if g < NG - 1:
                    for k in range(GRP):
                        jc = g * GRP + k
                        hp = build_hp(jc)
                        for it in range(8):
                            # start=True clears has_written for the WHOLE psum
                            # bank, so only the first slice sharing each bank
                            # may issue it; siblings then init via overwrite
                            # (has_written=0) on their first matmul.
                            nc.tensor.matmul(
                                accs[it][:],
                                gt[:, k, it * 128 : (it + 1) * 128],
                                hp[:],
                                start=(jc == 0 and it % 3 == 0),
                                stop=False,
                            )
                else:
                    # last group it-major: each bank's epilogue overlaps the
                    # remaining banks' matmuls. Bank order hops across PSUM
                    # banks (accs share banks in triples) so an epilogue's
                    # DVE reads never WAR-block the next bank's matmuls.
                    hps = [build_hp(g * GRP + k) for k in range(GRP)]
                    for it in (0, 3, 6, 1, 4, 7, 2, 5):
                        for k in range(GRP):
                            nc.tensor.matmul(
                                accs[it][:],
                                gt[:, k, it * 128 : (it + 1) * 128],
                                hps[k][:],
                                start=False,
                                stop=(k == GRP - 1),
                            )
                        epilogue(it)
                    nc.scalar.dma_start(out[:], ot_all[:])

    nc.compile()
    return nc


def make_in_maps(graph_info, h, W1, b1, W2, b2):
    """Shard + lay out the full inputs for the 8 cores."""
    import ml_dtypes

    fp16 = np.float16
    fp8 = ml_dtypes.float8_e4m3

    h = np.asarray(h, np.float32)
    hT = np.ascontiguousarray(h.T).astype(fp16)                # [D, N]
    hcm = np.ascontiguousarray(
        np.concatenate(
            [h.reshape(JC, 128, D), np.ones((JC, 128, 1), np.float32)], axis=2
        ).transpose(1, 0, 2)                                   # [128, JC, D+1]
    ).astype(fp16)
    W1s = np.asarray(W1, np.float32).astype(fp16)              # [D, HID]
    b1bt = np.ascontiguousarray(
        np.broadcast_to(np.asarray(b1, np.float32), (128, GRP, HID))
    ).astype(fp16)
    w2bt = np.ascontiguousarray(
        np.broadcast_to(np.asarray(W2, np.float32)[:, 0], (128, GRP, HID))
    ).astype(fp16)
    eb = np.full((128, 1), float(np.asarray(b2).reshape(-1)[0]) + ESHIFT,
                 np.float32)

    g8 = np.asarray(graph_info, np.float32).astype(fp8)        # exact 0/1
    in_maps = []
    for c in range(N_CORES):
        shard = g8[c * ROWS : (c + 1) * ROWS]                  # [1024, N]
        gTr = np.ascontiguousarray(
            shard.reshape(ROWS, JC, 128).transpose(2, 1, 0)    # [128, JC, 1024]
        )
        in_maps.append(
            {
                "gTr": gTr,
                "hT": hT,
                "hc": hcm,
                "W1s": W1s,
                "b1bt": b1bt,
                "w2bt": w2bt,
                "ebias": eb,
            }
        )
    return in_maps


def kernel(graph_info, h, W1, b1, W2, b2):
    _install_axon_hooks_shim()
    from concourse.bass_utils import run_bass_kernel_spmd

    if "nc" not in _cache:
        _cache["nc"] = build_nc()
    nc = _cache["nc"]

    in_maps = make_in_maps(graph_info, h, W1, b1, W2, b2)
    res = run_bass_kernel_spmd(nc, in_maps, list(range(N_CORES)))
    return np.concatenate(
        [
            res.results[c]["out"].transpose(1, 0, 2).reshape(ROWS, D)
            for c in range(N_CORES)
        ],
        axis=0,
    ).astype(np.float32)
